# revision 48
# baseline (speedup 1.0000x reference)
"""Multi-head attention (b=8, c=512, t=1024, 8 heads, e=64) on 8 TRN2 cores.

Strategy: pure data-parallel over batch - each NeuronCore handles one batch
element; weights replicated; no collectives.

v2 schedule (vs v0 baseline at ~133us):
  - ScalarE does ONLY exp (64 x [128,1024] ACTIVATEs ~= 61us floor); all bias
    adds / casts moved to VectorE.
  - Scores and av matmul pairs for two consecutive key-chunks are emitted
    back-to-back in the same PE tile config so LDWEIGHTS shadow-loads during
    the prior stream (216ns spacing instead of ~335ns).
  - Softmax denominators: 4-way col-packed M=1 ones-matmuls (two chunks x two
    heads at tile cols 0/32/64/96 -> psl partitions 0/32/64/96), halving the
    denominator group count. The per-query chunk-parity halves are re-summed
    inside the finalize broadcast matmul via zero-padded K=65 / K=97 lhsT=ones
    reads of lbA/lbB (only partitions {0,64} / {32,96} are nonzero).
  - Quarter order th-major: (p,0) for p=0..3 then (p,1), so proj(:,th=0) runs
    as PE filler in the exp-gated slack and only proj(:,th=1) remains in the
    tail.
  - qk/v/proj GEMMs are woven as single-stream filler units on a static
    per-block schedule sized to the exp-gated PE slack.
  - PSUM: psc 2x[128,1024] (4 banks) + pav 1 + psl 1 + pms 2 = 8 banks.
"""

import numpy as np
import ml_dtypes

_CACHE = {}

B, C, T = 8, 512, 1024
NH, E = 8, 64


def _build():
    import concourse.tile as tile
    from concourse import bacc, mybir

    F32 = mybir.dt.float32
    BF16 = mybir.dt.bfloat16
    AF = mybir.ActivationFunctionType

    nc = bacc.Bacc()

    x_d = nc.declare_dram_parameter("x", [C, T], BF16, isOutput=False)
    wqkv_d = nc.declare_dram_parameter("wqkv", [C, 3 * C], BF16, isOutput=False)
    # startup-critical weights, host-packed contiguous so one DMA dispatch
    # covers them: per c-block, [wq o-block 0 | wk o-block 4]
    wcrit_d = nc.declare_dram_parameter("wcrit", [128, 1024], BF16, isOutput=False)
    wproj_d = nc.declare_dram_parameter("wproj", [C, C], BF16, isOutput=False)
    bqk_d = nc.declare_dram_parameter("bqk", [128, 8], F32, isOutput=False)
    bvp_d = nc.declare_dram_parameter("bvp", [128, 4], F32, isOutput=False)
    bproj_d = nc.declare_dram_parameter("bproj", [128, 4], F32, isOutput=False)
    out_d = nc.declare_dram_parameter("out", [C, T], BF16, isOutput=True)

    with tile.TileContext(nc) as tc:
        with (
            tc.tile_pool(name="persist", bufs=1) as per,
            tc.tile_pool(name="ppool", bufs=1) as pp,
            tc.tile_pool(name="scr", bufs=2) as scr,
            tc.tile_pool(name="psc", bufs=2, space="PSUM") as psc,
            tc.tile_pool(name="pav", bufs=1, space="PSUM") as pav,
            tc.tile_pool(name="psl", bufs=1, space="PSUM") as pslp,
            tc.tile_pool(name="pms", bufs=2, space="PSUM") as pms,
        ):
            # ---- ones + PE warmup first (no input deps): release the HAM
            # clock gate while input DMAs land; also pre-trigger the exp
            # ACT table load with a tiny dummy activation.
            ones_all = per.tile([128, 512], BF16, tag="ones_all")
            nc.gpsimd.memset(ones_all[:], 1.0)
            warm = scr.tile([128, 8], F32, tag="wrm", name="warm")
            nc.scalar.activation(warm[:, 0:1], ones_all[:, 0:1], AF.Exp)
            pswarm = pms.tile([128, 512], F32, tag="ms", name="pswarm")
            for i in range(6):
                nc.tensor.matmul(
                    pswarm[:, 0:256], lhsT=ones_all[:, 0:128], rhs=ones_all[:, 0:256],
                    start=True, stop=True,
                )



            # ---- input DMAs, priority-ordered for the startup critical
            # path: x halves + wq o-block 0 + wk o-block 4 gate the first
            # scores; wv gates the v-GEMM fillers of the first blocks.
            # x as full [128,1024] tiles (2KB/partition DMA lines) spread
            # over four queues; small critical weight slices (wq o-block 0,
            # wk o-block 4, bqk) lead the scalar queue.
            bqk = per.tile([128, 8], F32, tag="bqk")
            xs = [per.tile([128, T], BF16, tag=f"x{c}", name=f"x{c}") for c in range(4)]
            wqkv = [
                per.tile([128, 3 * C], BF16, tag=f"wqkv{c}", name=f"wqkv{c}")
                for c in range(4)
            ]
            wcrit = per.tile([128, 1024], BF16, tag="wcrit")
            nc.sync.dma_start(out=xs[0][:], in_=x_d[0:128, :])
            nc.scalar.dma_start(out=wcrit[:], in_=wcrit_d[:, :])
            nc.gpsimd.dma_start(out=xs[3][:], in_=x_d[384:512, :])
            nc.sync.dma_start(out=xs[1][:], in_=x_d[128:256, :])
            nc.scalar.dma_start(out=bqk[:], in_=bqk_d[:, :])
            nc.scalar.dma_start(out=xs[2][:], in_=x_d[256:384, :])
            for c in range(4):  # wv (full, for v-GEMM fillers)
                nc.gpsimd.dma_start(
                    out=wqkv[c][:, 2 * C : 3 * C],
                    in_=wqkv_d[128 * c : 128 * (c + 1), 2 * C : 3 * C],
                )
            for c in range(4):  # wk o-blocks 5-7
                nc.sync.dma_start(
                    out=wqkv[c][:, 640:1024], in_=wqkv_d[128 * c : 128 * (c + 1), 640:1024]
                )
            for c in range(4):  # wq o-blocks 1-3
                nc.scalar.dma_start(
                    out=wqkv[c][:, 128:512], in_=wqkv_d[128 * c : 128 * (c + 1), 128:512]
                )
            bvp = per.tile([128, 4], F32, tag="bvp")
            nc.gpsimd.dma_start(out=bvp[:], in_=bvp_d[:, :])
            wproj = []
            for c in range(4):
                w = per.tile([128, C], BF16, tag=f"wproj{c}", name=f"wproj{c}")
                nc.gpsimd.dma_start(out=w[:], in_=wproj_d[128 * c : 128 * (c + 1), :])
                wproj.append(w)
            bproj = per.tile([128, 4], F32, tag="bproj")
            nc.gpsimd.dma_start(out=bproj[:], in_=bproj_d[:, :])

            # ---- persistent activations ----
            qk = [per.tile([128, T], BF16, tag=f"qk{o}", name=f"qk{o}") for o in range(8)]
            vT = [per.tile([128, C], BF16, tag=f"vT{t}", name=f"vT{t}") for t in range(8)]
            attnout = [
                per.tile([128, T], BF16, tag=f"ao{p}", name=f"ao{p}") for p in range(4)
            ]
            out_sb = [
                per.tile([128, T], BF16, tag=f"os{o}", name=f"os{o}") for o in range(4)
            ]

            # ---------- emitters ----------
            qk_ps = {}

            def qk_unit(o, half, c):
                # one c-pass of the (o, half) qk GEMM; accumulates in a pms
                # tile, bias-add+cast on VectorE at the last pass.
                tsl = slice(512 * half, 512 * (half + 1))
                if c == 0:
                    qk_ps[(o, half)] = pms.tile(
                        [128, 512], F32, tag="ms", name=f"psqk{o}_{half}"
                    )
                ps = qk_ps[(o, half)]
                if o == 0:
                    lhsT = wcrit[:, 256 * c : 256 * c + 128]
                elif o == 4:
                    lhsT = wcrit[:, 256 * c + 128 : 256 * c + 256]
                else:
                    lhsT = wqkv[c][:, 128 * o : 128 * (o + 1)]
                nc.tensor.matmul(
                    ps[:],
                    lhsT=lhsT,
                    rhs=xs[c][:, tsl],
                    start=(c == 0),
                    stop=(c == 3),
                )
                if c == 3:
                    nc.vector.tensor_scalar_add(qk[o][:, tsl], ps[:], bqk[:, o : o + 1])

            v_ps = {}

            def v_unit(tt, c):
                if c == 0:
                    v_ps[tt] = pms.tile([128, 512], F32, tag="ms", name=f"psv{tt}")
                ps = v_ps[tt]
                nc.tensor.matmul(
                    ps[:],
                    lhsT=xs[c][:, 128 * tt : 128 * (tt + 1)],
                    rhs=wqkv[c][:, 2 * C : 3 * C],
                    start=(c == 0),
                    stop=(c == 3),
                )
                if c == 3:
                    nc.vector.tensor_copy(vT[tt][:], ps[:])

            def sc_exp(p, th, u, g8):
                # scores chunk u (two heads row-packed) + exp on ScalarE
                tsl = slice(512 * th, 512 * (th + 1))
                ksl = slice(128 * u, 128 * (u + 1))
                ps = psc.tile([128, T], F32, tag="sc", name=f"pssc{p}_{th}_{u}")
                nc.tensor.matmul(
                    ps[:, 0:512],
                    lhsT=qk[4 + p][0:64, ksl], rhs=qk[p][0:64, tsl],
                    start=True, stop=True, tile_position=(0, 0),
                )
                nc.tensor.matmul(
                    ps[:, 512:1024],
                    lhsT=qk[4 + p][64:128, ksl], rhs=qk[p][64:128, tsl],
                    start=True, stop=True, tile_position=(64, 0),
                )
                P = pp.tile([128, T], BF16, tag=f"P{g8}{u % 2}",
                            name=f"P{p}_{th}_{u}")
                nc.scalar.activation(P[:], ps[:], AF.Exp)
                return P

            def av_chunk(p, u, P, psav):
                hA, hB = 2 * p, 2 * p + 1
                nc.tensor.matmul(
                    psav[0:64, :],
                    lhsT=vT[u][:, 64 * hA : 64 * (hA + 1)], rhs=P[:, 0:512],
                    start=(u == 0), stop=(u == 7), tile_position=(0, 0),
                )
                nc.tensor.matmul(
                    psav[64:128, :],
                    lhsT=vT[u][:, 64 * hB : 64 * (hB + 1)], rhs=P[:, 512:1024],
                    start=(u == 0), stop=(u == 7), tile_position=(0, 64),
                )

            def l4(j, Pe, Po, psl):
                # denominator sums, M=64-replicated ones-matmuls in the SAME
                # col-64 tile config as av (shadow weight loads, no config
                # switch): psl[0:64] = headA denom broadcast, psl[64:128] =
                # headB. Even+odd chunks accumulate into the same partitions.
                nc.tensor.matmul(
                    psl[0:64, :], lhsT=ones_all[:, 0:64], rhs=Pe[:, 0:512],
                    start=(j == 0), stop=False, tile_position=(0, 0),
                )
                nc.tensor.matmul(
                    psl[64:128, :], lhsT=ones_all[:, 0:64], rhs=Pe[:, 512:1024],
                    start=(j == 0), stop=False, tile_position=(0, 64),
                )
                nc.tensor.matmul(
                    psl[0:64, :], lhsT=ones_all[:, 0:64], rhs=Po[:, 0:512],
                    start=False, stop=(j == 3), tile_position=(0, 0),
                )
                nc.tensor.matmul(
                    psl[64:128, :], lhsT=ones_all[:, 0:64], rhs=Po[:, 512:1024],
                    start=False, stop=(j == 3), tile_position=(0, 64),
                )

            def finalize_head(p, th, psav, psl):
                # free pav / psl promptly: the ScalarE copy and the reciprocal
                # are the only PSUM readers, so the next quarter's av/l4
                # matmuls don't wait on the rest of the chain.
                avc = scr.tile([128, 512], F32, tag="avc", name=f"avc{p}_{th}")
                nc.scalar.copy(avc[:], psav[:])
                bc = scr.tile([128, 512], F32, tag="bc", name=f"bc{p}_{th}")
                nc.vector.reciprocal_approx_fast(bc[:], psl[:])
                return avc, bc

            def finalize_tail(p, th, avc, bc):
                tsl = slice(512 * th, 512 * (th + 1))
                tmpn = scr.tile([128, 512], F32, tag="tmpn", name=f"tn{p}_{th}")
                nc.vector.tensor_mul(tmpn[:], avc[:], bc[:])
                nc.vector.tensor_scalar_add(
                    attnout[p][:, tsl], tmpn[:], bvp[:, p : p + 1]
                )

            proj_ps = {}

            def proj_unit(o, th, c):
                tsl = slice(512 * th, 512 * (th + 1))
                if c == 0:
                    proj_ps[(o, th)] = pms.tile(
                        [128, 512], F32, tag="ms", name=f"pspr{o}_{th}"
                    )
                ps = proj_ps[(o, th)]
                nc.tensor.matmul(
                    ps[:],
                    lhsT=wproj[c][:, 128 * o : 128 * (o + 1)],
                    rhs=attnout[c][:, tsl],
                    start=(c == 0),
                    stop=(c == 3),
                )
                if c == 3:
                    nc.vector.tensor_scalar_add(
                        out_sb[o][:, tsl], ps[:], bproj[:, o : o + 1]
                    )
                    eng = [nc.sync, nc.gpsimd, nc.scalar, nc.sync][o]
                    eng.dma_start(
                        out=out_d[128 * o : 128 * (o + 1), tsl], in_=out_sb[o][:, tsl]
                    )

            # ---------- static filler schedule ----------
            def qk_full(o, half):
                return [lambda c=c: qk_unit(o, half, c) for c in range(4)]

            def v_full(tt):
                return [lambda c=c: v_unit(tt, c) for c in range(4)]

            def proj_full(o, th):
                return [lambda c=c: proj_unit(o, th, c) for c in range(4)]

            # fillers[b] = (pre_av_units, post_av_units). Deadlines:
            #   vT[u] before the av that consumes it (av at block b covers
            #   chunks of pair b-LAG); qk halves before the sc that reads
            #   them; proj(:,0) after finalize(q3) pops at block 21.
            fillers = {
                0: ([], qk_full(4, 1)[:2] + v_full(0)),
                1: ([], qk_full(4, 1)[2:] + v_full(1)),
                2: ([], v_full(2) + qk_full(5, 0)[:2]),
                3: ([], qk_full(5, 0)[2:] + qk_full(1, 0)),
                4: (v_full(3), v_full(4) + qk_full(5, 1)[:2]),
                5: (v_full(5), qk_full(5, 1)[2:] + v_full(6)[:2]),
                6: (v_full(6)[2:] + v_full(7), qk_full(6, 0)[:2]),
                7: ([], qk_full(6, 0)[2:] + qk_full(2, 0)),
                8: ([], qk_full(6, 1)[:2]),
                9: ([], qk_full(6, 1)[2:] + qk_full(7, 0)[:2]),
                10: ([], qk_full(7, 0)[2:] + qk_full(3, 0)[:2]),
                11: ([], qk_full(3, 0)[2:] + qk_full(7, 1)[:2]),
                12: ([], qk_full(7, 1)[2:] + qk_full(0, 1)[:2]),
                13: ([], qk_full(0, 1)[2:] + qk_full(1, 1)[:2]),
                14: ([], qk_full(1, 1)[2:] + qk_full(2, 1)[:2]),
                15: ([], qk_full(2, 1)[2:] + qk_full(3, 1)[:2]),
                16: ([], qk_full(3, 1)[2:]),
                19: ([], proj_full(0, 0)),
                20: ([], proj_full(1, 0)),
                21: ([], proj_full(2, 0)),
                22: ([], proj_full(3, 0)),
            }

            # ---------- startup PE work ----------
            psl_t = pslp.tile([128, 512], F32, tag="l", name="psl")
            for f in qk_full(0, 0) + qk_full(4, 0):
                f()

            # ---------- main pipeline ----------
            LAG = 3
            quarters = [(p, th) for th in range(2) for p in range(4)]
            pend = []          # (p, th, j, P_even, P_odd)
            qtiles = {}        # (p, th) -> psav

            fin_q = []

            def emit_avl(ent):
                p, th, j, Pe, Po = ent
                if (p, th) not in qtiles:
                    qtiles[(p, th)] = pav.tile(
                        [128, 512], F32, tag="av", name=f"psav{p}_{th}"
                    )
                psav = qtiles[(p, th)]
                av_chunk(p, 2 * j, Pe, psav)
                av_chunk(p, 2 * j + 1, Po, psav)
                l4(j, Pe, Po, psl_t)
                if j == 3:
                    avc, lbq = finalize_head(p, th, psav, psl_t)
                    fin_q.append(lambda a=avc, b=lbq, pp_=p, tt=th:
                                 finalize_tail(pp_, tt, a, b))

            # drain the av/l pipeline early over the last quarters' blocks so
            # the tail holds only the final finalize + proj(:,1).
            drain_target = {28: 3, 29: 2, 30: 1, 31: 0}
            for b in range(32):
                qi, j = b // 4, b % 4
                p, th = quarters[qi]
                pre, post = fillers.get(b, ([], []))
                for f in pre:
                    f()
                Pe = sc_exp(p, th, 2 * j, b % 4)
                Po = sc_exp(p, th, 2 * j + 1, b % 4)
                pend.append((p, th, j, Pe, Po))
                while fin_q:
                    fin_q.pop(0)()
                while len(pend) > drain_target.get(b, LAG):
                    emit_avl(pend.pop(0))
                for f in post:
                    f()
            while pend:
                emit_avl(pend.pop(0))
            while fin_q:
                fin_q.pop(0)()
            for o in range(4):
                for f in proj_full(o, 1):
                    f()

    nc.compile()
    return nc


def _get_nc():
    if "nc" not in _CACHE:
        _CACHE["nc"] = _build()
    return _CACHE["nc"]


def kernel(x, qkv_w, qkv_b, proj_w, proj_b, _trace=False):
    from concourse.bass_utils import run_bass_kernel_spmd

    nc = _get_nc()

    bf16 = ml_dtypes.bfloat16
    b, c, h, w = x.shape
    xf = np.asarray(x, dtype=np.float32).reshape(b, c, h * w)
    qkv_b = np.asarray(qkv_b, dtype=np.float32)
    qkv_w = np.asarray(qkv_w, dtype=np.float32)
    # fold the 1/sqrt(e)=1/8 softmax scale into Wq / bq on host
    qkv_w = np.concatenate([qkv_w[:512] * 0.125, qkv_w[512:]], axis=0)
    bq = np.concatenate([qkv_b[:512] * 0.125, qkv_b[512:1024]])
    wqkvT = np.ascontiguousarray(qkv_w.T).astype(bf16)
    wprojT = np.ascontiguousarray(np.asarray(proj_w, np.float32).T).astype(bf16)
    bqk = np.ascontiguousarray(bq.reshape(8, 128).T)
    bvp = np.ascontiguousarray(qkv_b[1024:1536].reshape(4, 128).T)
    bproj = np.ascontiguousarray(np.asarray(proj_b, np.float32).reshape(4, 128).T)

    wcrit = np.concatenate(
        [
            np.concatenate(
                [wqkvT[128 * c : 128 * (c + 1), 0:128],
                 wqkvT[128 * c : 128 * (c + 1), 512:640]],
                axis=1,
            )
            for c in range(4)
        ],
        axis=1,
    )
    wcrit = np.ascontiguousarray(wcrit)

    in_maps = [
        dict(
            x=np.ascontiguousarray(xf[i]).astype(bf16),
            wqkv=wqkvT,
            wcrit=wcrit,
            wproj=wprojT,
            bqk=bqk,
            bvp=bvp,
            bproj=bproj,
        )
        for i in range(b)
    ]
    res = run_bass_kernel_spmd(nc, in_maps, core_ids=list(range(8)), trace=_trace)
    out = np.stack([res.results[i]["out"].astype(np.float32) for i in range(b)])
    out = out.reshape(b, c, h, w)
    if _trace:
        _CACHE["last_result"] = res
    return out


# revision 50
# speedup vs baseline: 1.1007x; 1.1007x over previous
"""Multi-head attention (b=8, c=512, t=1024, 8 heads, e=64) on 8 TRN2 cores.

Strategy: pure data-parallel over batch - each NeuronCore handles one batch
element; weights replicated; no collectives.

v2 schedule (vs v0 baseline at ~133us):
  - ScalarE does ONLY exp (64 x [128,1024] ACTIVATEs ~= 61us floor); all bias
    adds / casts moved to VectorE.
  - Scores and av matmul pairs for two consecutive key-chunks are emitted
    back-to-back in the same PE tile config so LDWEIGHTS shadow-loads during
    the prior stream (216ns spacing instead of ~335ns).
  - Softmax denominators: 4-way col-packed M=1 ones-matmuls (two chunks x two
    heads at tile cols 0/32/64/96 -> psl partitions 0/32/64/96), halving the
    denominator group count. The per-query chunk-parity halves are re-summed
    inside the finalize broadcast matmul via zero-padded K=65 / K=97 lhsT=ones
    reads of lbA/lbB (only partitions {0,64} / {32,96} are nonzero).
  - Quarter order th-major: (p,0) for p=0..3 then (p,1), so proj(:,th=0) runs
    as PE filler in the exp-gated slack and only proj(:,th=1) remains in the
    tail.
  - qk/v/proj GEMMs are woven as single-stream filler units on a static
    per-block schedule sized to the exp-gated PE slack.
  - PSUM: psc 2x[128,1024] (4 banks) + pav 1 + psl 1 + pms 2 = 8 banks.
"""

import numpy as np
import ml_dtypes

_CACHE = {}

B, C, T = 8, 512, 1024
NH, E = 8, 64


def _build():
    import concourse.tile as tile
    from concourse import bacc, mybir

    F32 = mybir.dt.float32
    BF16 = mybir.dt.bfloat16
    AF = mybir.ActivationFunctionType

    nc = bacc.Bacc()

    x_d = nc.declare_dram_parameter("x", [C, T], BF16, isOutput=False)
    wqkv_d = nc.declare_dram_parameter("wqkv", [C, 3 * C], BF16, isOutput=False)
    # startup-critical weights, host-packed contiguous so one DMA dispatch
    # covers them: per c-block, [wq o-block 0 | wk o-block 4]
    wcrit_d = nc.declare_dram_parameter("wcrit", [128, 1024], BF16, isOutput=False)
    wproj_d = nc.declare_dram_parameter("wproj", [C, C], BF16, isOutput=False)
    bqk_d = nc.declare_dram_parameter("bqk", [128, 8], F32, isOutput=False)
    bvp_d = nc.declare_dram_parameter("bvp", [128, 4], F32, isOutput=False)
    bproj_d = nc.declare_dram_parameter("bproj", [128, 4], F32, isOutput=False)
    out_d = nc.declare_dram_parameter("out", [C, T], BF16, isOutput=True)

    with tile.TileContext(nc) as tc:
        with (
            tc.tile_pool(name="persist", bufs=1) as per,
            tc.tile_pool(name="ppool", bufs=1) as pp,
            tc.tile_pool(name="scr", bufs=2) as scr,
            tc.tile_pool(name="psc", bufs=2, space="PSUM") as psc,
            tc.tile_pool(name="pav", bufs=1, space="PSUM") as pav,
            tc.tile_pool(name="psl", bufs=1, space="PSUM") as pslp,
            tc.tile_pool(name="pms", bufs=2, space="PSUM") as pms,
        ):
            # ---- ones + PE warmup first (no input deps): release the HAM
            # clock gate while input DMAs land; also pre-trigger the exp
            # ACT table load with a tiny dummy activation.
            ones_all = per.tile([128, 512], BF16, tag="ones_all")
            nc.gpsimd.memset(ones_all[:], 1.0)
            warm = scr.tile([128, 8], F32, tag="wrm", name="warm")
            nc.scalar.activation(warm[:, 0:1], ones_all[:, 0:1], AF.Exp)
            pswarm = pms.tile([128, 512], F32, tag="ms", name="pswarm")
            for i in range(6):
                nc.tensor.matmul(
                    pswarm[:, 0:256], lhsT=ones_all[:, 0:128], rhs=ones_all[:, 0:256],
                    start=True, stop=True,
                )



            # ---- input DMAs, priority-ordered for the startup critical
            # path: x halves + wq o-block 0 + wk o-block 4 gate the first
            # scores; wv gates the v-GEMM fillers of the first blocks.
            # x as full [128,1024] tiles (2KB/partition DMA lines) spread
            # over four queues; small critical weight slices (wq o-block 0,
            # wk o-block 4, bqk) lead the scalar queue.
            bqk = per.tile([128, 8], F32, tag="bqk")
            xs = [per.tile([128, T], BF16, tag=f"x{c}", name=f"x{c}") for c in range(4)]
            wqkv = [
                per.tile([128, 3 * C], BF16, tag=f"wqkv{c}", name=f"wqkv{c}")
                for c in range(4)
            ]
            wcrit = per.tile([128, 1024], BF16, tag="wcrit")
            nc.sync.dma_start(out=xs[0][:], in_=x_d[0:128, :])
            nc.scalar.dma_start(out=wcrit[:], in_=wcrit_d[:, :])
            nc.gpsimd.dma_start(out=xs[3][:], in_=x_d[384:512, :])
            nc.sync.dma_start(out=xs[1][:], in_=x_d[128:256, :])
            nc.scalar.dma_start(out=bqk[:], in_=bqk_d[:, :])
            nc.scalar.dma_start(out=xs[2][:], in_=x_d[256:384, :])
            for c in range(4):  # wv (full, for v-GEMM fillers)
                nc.gpsimd.dma_start(
                    out=wqkv[c][:, 2 * C : 3 * C],
                    in_=wqkv_d[128 * c : 128 * (c + 1), 2 * C : 3 * C],
                )
            for c in range(4):  # wk o-blocks 5-7
                nc.sync.dma_start(
                    out=wqkv[c][:, 640:1024], in_=wqkv_d[128 * c : 128 * (c + 1), 640:1024]
                )
            for c in range(4):  # wq o-blocks 1-3
                nc.scalar.dma_start(
                    out=wqkv[c][:, 128:512], in_=wqkv_d[128 * c : 128 * (c + 1), 128:512]
                )
            bvp = per.tile([128, 4], F32, tag="bvp")
            nc.gpsimd.dma_start(out=bvp[:], in_=bvp_d[:, :])
            wproj = []
            for c in range(4):
                w = per.tile([128, C], BF16, tag=f"wproj{c}", name=f"wproj{c}")
                nc.gpsimd.dma_start(out=w[:], in_=wproj_d[128 * c : 128 * (c + 1), :])
                wproj.append(w)
            bproj = per.tile([128, 4], F32, tag="bproj")
            nc.gpsimd.dma_start(out=bproj[:], in_=bproj_d[:, :])

            # ---- persistent activations ----
            qk = [per.tile([128, T], BF16, tag=f"qk{o}", name=f"qk{o}") for o in range(8)]
            vT = [per.tile([128, C], BF16, tag=f"vT{t}", name=f"vT{t}") for t in range(8)]
            attnout = [
                per.tile([128, T], BF16, tag=f"ao{p}", name=f"ao{p}") for p in range(4)
            ]
            out_sb = [
                per.tile([128, T], BF16, tag=f"os{o}", name=f"os{o}") for o in range(4)
            ]

            # ---------- emitters ----------
            qk_ps = {}

            def qk_unit(o, half, c):
                # one c-pass of the (o, half) qk GEMM; accumulates in a pms
                # tile, bias-add+cast on VectorE at the last pass.
                tsl = slice(512 * half, 512 * (half + 1))
                if c == 0:
                    qk_ps[(o, half)] = pms.tile(
                        [128, 512], F32, tag="ms", name=f"psqk{o}_{half}"
                    )
                ps = qk_ps[(o, half)]
                if o == 0:
                    lhsT = wcrit[:, 256 * c : 256 * c + 128]
                elif o == 4:
                    lhsT = wcrit[:, 256 * c + 128 : 256 * c + 256]
                else:
                    lhsT = wqkv[c][:, 128 * o : 128 * (o + 1)]
                nc.tensor.matmul(
                    ps[:],
                    lhsT=lhsT,
                    rhs=xs[c][:, tsl],
                    start=(c == 0),
                    stop=(c == 3),
                )
                if c == 3:
                    nc.vector.tensor_scalar_add(qk[o][:, tsl], ps[:], bqk[:, o : o + 1])

            v_ps = {}

            def v_unit(tt, c):
                if c == 0:
                    v_ps[tt] = pms.tile([128, 512], F32, tag="ms", name=f"psv{tt}")
                ps = v_ps[tt]
                nc.tensor.matmul(
                    ps[:],
                    lhsT=xs[c][:, 128 * tt : 128 * (tt + 1)],
                    rhs=wqkv[c][:, 2 * C : 3 * C],
                    start=(c == 0),
                    stop=(c == 3),
                )
                if c == 3:
                    nc.vector.tensor_copy(vT[tt][:], ps[:])

            def sc_exp(p, th, u, g8):
                # scores chunk u (two heads row-packed) + exp on ScalarE
                tsl = slice(512 * th, 512 * (th + 1))
                ksl = slice(128 * u, 128 * (u + 1))
                ps = psc.tile([128, T], F32, tag="sc", name=f"pssc{p}_{th}_{u}")
                nc.tensor.matmul(
                    ps[:, 0:512],
                    lhsT=qk[4 + p][0:64, ksl], rhs=qk[p][0:64, tsl],
                    start=True, stop=True, tile_position=(0, 0),
                )
                nc.tensor.matmul(
                    ps[:, 512:1024],
                    lhsT=qk[4 + p][64:128, ksl], rhs=qk[p][64:128, tsl],
                    start=True, stop=True, tile_position=(64, 0),
                )
                P = pp.tile([128, T], BF16, tag=f"P{g8}{u % 2}",
                            name=f"P{p}_{th}_{u}")
                nc.scalar.activation(P[:], ps[:], AF.Exp)
                return P

            def av_chunk(p, u, P, psav):
                hA, hB = 2 * p, 2 * p + 1
                nc.tensor.matmul(
                    psav[0:64, :],
                    lhsT=vT[u][:, 64 * hA : 64 * (hA + 1)], rhs=P[:, 0:512],
                    start=(u == 0), stop=(u == 7), tile_position=(0, 0),
                )
                nc.tensor.matmul(
                    psav[64:128, :],
                    lhsT=vT[u][:, 64 * hB : 64 * (hB + 1)], rhs=P[:, 512:1024],
                    start=(u == 0), stop=(u == 7), tile_position=(0, 64),
                )

            def l4(j, Pe, Po, psl):
                # denominator partial sums for chunks (2j, 2j+1), both heads:
                # 4-way col-packed M=1 ones-matmuls. psl partitions:
                # 0 = headA/even-chunks, 32 = headA/odd, 64 = headB/even,
                # 96 = headB/odd; accumulate across the quarter's 4 blocks.
                st, sp = (j == 0), (j == 3)
                nc.tensor.matmul(
                    psl[0:1, :], lhsT=ones_all[:, 0:1], rhs=Pe[:, 0:512],
                    start=st, stop=sp, tile_position=(0, 0),
                )
                nc.tensor.matmul(
                    psl[64:65, :], lhsT=ones_all[:, 0:1], rhs=Pe[:, 512:1024],
                    start=st, stop=sp, tile_position=(0, 64),
                )
                nc.tensor.matmul(
                    psl[32:33, :], lhsT=ones_all[:, 0:1], rhs=Po[:, 0:512],
                    start=st, stop=sp, tile_position=(0, 32),
                )
                nc.tensor.matmul(
                    psl[96:97, :], lhsT=ones_all[:, 0:1], rhs=Po[:, 512:1024],
                    start=st, stop=sp, tile_position=(0, 96),
                )

            def finalize_head(p, th, psav, psl):
                # free pav / psl as early as possible: ScalarE copies run in
                # the natural exp-stream bubble at the quarter boundary, so
                # the next quarter's av/l4 matmuls don't queue on VectorE.
                avc = scr.tile([128, 512], F32, tag="avc", name=f"avc{p}_{th}")
                nc.scalar.copy(avc[:], psav[:])
                lbq = scr.tile([128, 512], BF16, tag="lb", name=f"lb{p}_{th}")
                nc.vector.tensor_copy(lbq[:], psl[:])
                return avc, lbq

            def finalize_tail(p, th, avc, lbq):
                tsl = slice(512 * th, 512 * (th + 1))
                # broadcast-and-parity-sum: K=33 ones-matmuls over padded row
                # ranges of lbq (rows 1-31, 33-63, 65-95, 97-127 are zero).
                psbc = pms.tile([128, 512], F32, tag="ms", name=f"psbc{p}_{th}")
                nc.tensor.matmul(
                    psbc[0:64, :], lhsT=ones_all[0:33, 0:64], rhs=lbq[0:33, :],
                    start=True, stop=True, tile_position=(0, 0),
                )
                nc.tensor.matmul(
                    psbc[64:128, :], lhsT=ones_all[64:97, 0:64], rhs=lbq[64:97, :],
                    start=True, stop=True, tile_position=(64, 64),
                )
                bc = scr.tile([128, 512], F32, tag="bc", name=f"bc{p}_{th}")
                nc.vector.reciprocal_approx_fast(bc[:], psbc[:])
                tmpn = scr.tile([128, 512], F32, tag="tmpn", name=f"tn{p}_{th}")
                nc.vector.tensor_mul(tmpn[:], avc[:], bc[:])
                nc.vector.tensor_scalar_add(
                    attnout[p][:, tsl], tmpn[:], bvp[:, p : p + 1]
                )

            proj_ps = {}

            def proj_unit(o, th, c):
                tsl = slice(512 * th, 512 * (th + 1))
                if c == 0:
                    proj_ps[(o, th)] = pms.tile(
                        [128, 512], F32, tag="ms", name=f"pspr{o}_{th}"
                    )
                ps = proj_ps[(o, th)]
                nc.tensor.matmul(
                    ps[:],
                    lhsT=wproj[c][:, 128 * o : 128 * (o + 1)],
                    rhs=attnout[c][:, tsl],
                    start=(c == 0),
                    stop=(c == 3),
                )
                if c == 3:
                    nc.vector.tensor_scalar_add(
                        out_sb[o][:, tsl], ps[:], bproj[:, o : o + 1]
                    )
                    eng = [nc.sync, nc.gpsimd, nc.scalar, nc.sync][o]
                    eng.dma_start(
                        out=out_d[128 * o : 128 * (o + 1), tsl], in_=out_sb[o][:, tsl]
                    )

            # ---------- static filler schedule ----------
            def qk_full(o, half):
                return [lambda c=c: qk_unit(o, half, c) for c in range(4)]

            def v_full(tt):
                return [lambda c=c: v_unit(tt, c) for c in range(4)]

            def proj_full(o, th):
                return [lambda c=c: proj_unit(o, th, c) for c in range(4)]

            # fillers[b] = (pre_av_units, post_av_units). Deadlines:
            #   vT[u] before the av that consumes it (av at block b covers
            #   chunks of pair b-LAG); qk halves before the sc that reads
            #   them; proj(:,0) after finalize(q3) pops at block 21.
            fillers = {
                0: ([], qk_full(4, 1)[:2] + v_full(0)),
                1: ([], qk_full(4, 1)[2:] + v_full(1)),
                2: ([], v_full(2) + qk_full(5, 0)[:2]),
                3: ([], qk_full(5, 0)[2:] + qk_full(1, 0)),
                4: (v_full(3), v_full(4) + qk_full(5, 1)[:2]),
                5: (v_full(5), qk_full(5, 1)[2:] + v_full(6)[:2]),
                6: (v_full(6)[2:] + v_full(7), qk_full(6, 0)[:2]),
                7: ([], qk_full(6, 0)[2:] + qk_full(2, 0)),
                8: ([], qk_full(6, 1)[:2]),
                9: ([], qk_full(6, 1)[2:] + qk_full(7, 0)[:2]),
                10: ([], qk_full(7, 0)[2:] + qk_full(3, 0)[:2]),
                11: ([], qk_full(3, 0)[2:] + qk_full(7, 1)[:2]),
                12: ([], qk_full(7, 1)[2:] + qk_full(0, 1)[:2]),
                13: ([], qk_full(0, 1)[2:] + qk_full(1, 1)[:2]),
                14: ([], qk_full(1, 1)[2:] + qk_full(2, 1)[:2]),
                15: ([], qk_full(2, 1)[2:] + qk_full(3, 1)[:2]),
                16: ([], qk_full(3, 1)[2:]),
                19: ([], proj_full(0, 0)),
                20: ([], proj_full(1, 0)),
                21: ([], proj_full(2, 0)),
                22: ([], proj_full(3, 0)),
            }

            # ---------- startup PE work ----------
            # single persistent psl tile; zero its never-written partitions
            # once so the padded K=33 broadcast reads are defined.
            psl_t = pslp.tile([128, 512], F32, tag="l", name="psl")
            nc.vector.memset(psl_t[:], 0.0)
            for f in qk_full(0, 0) + qk_full(4, 0):
                f()

            # ---------- main pipeline ----------
            LAG = 3
            quarters = [(p, th) for th in range(2) for p in range(4)]
            pend = []          # (p, th, j, P_even, P_odd)
            qtiles = {}        # (p, th) -> psav

            fin_q = []

            def emit_avl(ent):
                p, th, j, Pe, Po = ent
                if (p, th) not in qtiles:
                    qtiles[(p, th)] = pav.tile(
                        [128, 512], F32, tag="av", name=f"psav{p}_{th}"
                    )
                psav = qtiles[(p, th)]
                av_chunk(p, 2 * j, Pe, psav)
                av_chunk(p, 2 * j + 1, Po, psav)
                l4(j, Pe, Po, psl_t)
                if j == 3:
                    avc, lbq = finalize_head(p, th, psav, psl_t)
                    fin_q.append(lambda a=avc, b=lbq, pp_=p, tt=th:
                                 finalize_tail(pp_, tt, a, b))

            # drain the av/l pipeline early over the last quarters' blocks so
            # the tail holds only the final finalize + proj(:,1).
            drain_target = {28: 3, 29: 2, 30: 1, 31: 0}
            for b in range(32):
                qi, j = b // 4, b % 4
                p, th = quarters[qi]
                pre, post = fillers.get(b, ([], []))
                for f in pre:
                    f()
                Pe = sc_exp(p, th, 2 * j, b % 4)
                Po = sc_exp(p, th, 2 * j + 1, b % 4)
                pend.append((p, th, j, Pe, Po))
                while fin_q:
                    fin_q.pop(0)()
                while len(pend) > drain_target.get(b, LAG):
                    emit_avl(pend.pop(0))
                for f in post:
                    f()
            while pend:
                emit_avl(pend.pop(0))
            while fin_q:
                fin_q.pop(0)()
            for o in range(4):
                for f in proj_full(o, 1):
                    f()

    nc.compile()
    return nc


def _get_nc():
    if "nc" not in _CACHE:
        _CACHE["nc"] = _build()
    return _CACHE["nc"]


def kernel(x, qkv_w, qkv_b, proj_w, proj_b, _trace=False):
    from concourse.bass_utils import run_bass_kernel_spmd

    nc = _get_nc()

    bf16 = ml_dtypes.bfloat16
    b, c, h, w = x.shape
    xf = np.asarray(x, dtype=np.float32).reshape(b, c, h * w)
    qkv_b = np.asarray(qkv_b, dtype=np.float32)
    qkv_w = np.asarray(qkv_w, dtype=np.float32)
    # fold the 1/sqrt(e)=1/8 softmax scale into Wq / bq on host
    qkv_w = np.concatenate([qkv_w[:512] * 0.125, qkv_w[512:]], axis=0)
    bq = np.concatenate([qkv_b[:512] * 0.125, qkv_b[512:1024]])
    wqkvT = np.ascontiguousarray(qkv_w.T).astype(bf16)
    wprojT = np.ascontiguousarray(np.asarray(proj_w, np.float32).T).astype(bf16)
    bqk = np.ascontiguousarray(bq.reshape(8, 128).T)
    bvp = np.ascontiguousarray(qkv_b[1024:1536].reshape(4, 128).T)
    bproj = np.ascontiguousarray(np.asarray(proj_b, np.float32).reshape(4, 128).T)

    wcrit = np.concatenate(
        [
            np.concatenate(
                [wqkvT[128 * c : 128 * (c + 1), 0:128],
                 wqkvT[128 * c : 128 * (c + 1), 512:640]],
                axis=1,
            )
            for c in range(4)
        ],
        axis=1,
    )
    wcrit = np.ascontiguousarray(wcrit)

    in_maps = [
        dict(
            x=np.ascontiguousarray(xf[i]).astype(bf16),
            wqkv=wqkvT,
            wcrit=wcrit,
            wproj=wprojT,
            bqk=bqk,
            bvp=bvp,
            bproj=bproj,
        )
        for i in range(b)
    ]
    res = run_bass_kernel_spmd(nc, in_maps, core_ids=list(range(8)), trace=_trace)
    out = np.stack([res.results[i]["out"].astype(np.float32) for i in range(b)])
    out = out.reshape(b, c, h, w)
    if _trace:
        _CACHE["last_result"] = res
    return out


# revision 54
# speedup vs baseline: 1.1480x; 1.0430x over previous
"""Multi-head attention (b=8, c=512, t=1024, 8 heads, e=64) on 8 TRN2 cores.

Strategy: pure data-parallel over batch - each NeuronCore handles one batch
element; weights replicated; no collectives.

v2 schedule (vs v0 baseline at ~133us):
  - ScalarE does ONLY exp (64 x [128,1024] ACTIVATEs ~= 61us floor); all bias
    adds / casts moved to VectorE.
  - Scores and av matmul pairs for two consecutive key-chunks are emitted
    back-to-back in the same PE tile config so LDWEIGHTS shadow-loads during
    the prior stream (216ns spacing instead of ~335ns).
  - Softmax denominators: 4-way col-packed M=1 ones-matmuls (two chunks x two
    heads at tile cols 0/32/64/96 -> psl partitions 0/32/64/96), halving the
    denominator group count. The per-query chunk-parity halves are re-summed
    inside the finalize broadcast matmul via zero-padded K=65 / K=97 lhsT=ones
    reads of lbA/lbB (only partitions {0,64} / {32,96} are nonzero).
  - Quarter order th-major: (p,0) for p=0..3 then (p,1), so proj(:,th=0) runs
    as PE filler in the exp-gated slack and only proj(:,th=1) remains in the
    tail.
  - qk/v/proj GEMMs are woven as single-stream filler units on a static
    per-block schedule sized to the exp-gated PE slack.
  - PSUM: psc 2x[128,1024] (4 banks) + pav 1 + psl 1 + pms 2 = 8 banks.
"""

import numpy as np
import ml_dtypes

_CACHE = {}

B, C, T = 8, 512, 1024
NH, E = 8, 64


def _build():
    import concourse.tile as tile
    from concourse import bacc, mybir

    F32 = mybir.dt.float32
    BF16 = mybir.dt.bfloat16
    AF = mybir.ActivationFunctionType

    nc = bacc.Bacc()

    x_d = nc.declare_dram_parameter("x", [C, T], BF16, isOutput=False)
    wqkv_d = nc.declare_dram_parameter("wqkv", [C, 3 * C], BF16, isOutput=False)
    # startup-critical weights, host-packed contiguous so one DMA dispatch
    # covers them: per c-block, [wq o-block 0 | wk o-block 4]
    wcrit_d = nc.declare_dram_parameter("wcrit", [128, 1024], BF16, isOutput=False)
    wproj_d = nc.declare_dram_parameter("wproj", [C, C], BF16, isOutput=False)
    bqk_d = nc.declare_dram_parameter("bqk", [128, 8], F32, isOutput=False)
    bvp_d = nc.declare_dram_parameter("bvp", [128, 4], F32, isOutput=False)
    bproj_d = nc.declare_dram_parameter("bproj", [128, 4], F32, isOutput=False)
    out_d = nc.declare_dram_parameter("out", [C, T], BF16, isOutput=True)

    with tile.TileContext(nc) as tc:
        with (
            tc.tile_pool(name="persist", bufs=1) as per,
            tc.tile_pool(name="ppool", bufs=1) as pp,
            tc.tile_pool(name="scr", bufs=2) as scr,
            tc.tile_pool(name="psc", bufs=2, space="PSUM") as psc,
            tc.tile_pool(name="pav", bufs=1, space="PSUM") as pav,
            tc.tile_pool(name="psl", bufs=1, space="PSUM") as pslp,
            tc.tile_pool(name="pms", bufs=2, space="PSUM") as pms,
        ):
            # ---- ones + PE warmup first (no input deps): release the HAM
            # clock gate while input DMAs land; also pre-trigger the exp
            # ACT table load with a tiny dummy activation.
            ones_all = per.tile([128, 512], BF16, tag="ones_all")
            nc.gpsimd.memset(ones_all[:], 1.0)
            warm = scr.tile([128, 8], F32, tag="wrm", name="warm")
            nc.scalar.activation(warm[:, 0:1], ones_all[:, 0:1], AF.Exp)
            pswarm = pms.tile([128, 512], F32, tag="ms", name="pswarm")
            for i in range(6):
                nc.tensor.matmul(
                    pswarm[:, 0:256], lhsT=ones_all[:, 0:128], rhs=ones_all[:, 0:256],
                    start=True, stop=True,
                )



            # ---- input DMAs, priority-ordered for the startup critical
            # path: x halves + wq o-block 0 + wk o-block 4 gate the first
            # scores; wv gates the v-GEMM fillers of the first blocks.
            # x as full [128,1024] tiles (2KB/partition DMA lines) spread
            # over four queues; small critical weight slices (wq o-block 0,
            # wk o-block 4, bqk) lead the scalar queue.
            bqk = per.tile([128, 8], F32, tag="bqk")
            xs = [per.tile([128, T], BF16, tag=f"x{c}", name=f"x{c}") for c in range(4)]
            wqkv = [
                per.tile([128, 3 * C], BF16, tag=f"wqkv{c}", name=f"wqkv{c}")
                for c in range(4)
            ]
            wcrit = per.tile([128, 1024], BF16, tag="wcrit")
            nc.sync.dma_start(out=xs[0][:], in_=x_d[0:128, :])
            nc.scalar.dma_start(out=wcrit[:], in_=wcrit_d[:, :])
            nc.gpsimd.dma_start(out=xs[3][:], in_=x_d[384:512, :])
            nc.sync.dma_start(out=xs[1][:], in_=x_d[128:256, :])
            nc.scalar.dma_start(out=xs[2][:], in_=x_d[256:384, :])
            nc.scalar.dma_start(out=bqk[:], in_=bqk_d[:, :])
            for c in range(4):  # wv (full, for v-GEMM fillers)
                nc.gpsimd.dma_start(
                    out=wqkv[c][:, 2 * C : 3 * C],
                    in_=wqkv_d[128 * c : 128 * (c + 1), 2 * C : 3 * C],
                )
            for c in range(4):  # wk o-blocks 5-7
                nc.sync.dma_start(
                    out=wqkv[c][:, 640:1024], in_=wqkv_d[128 * c : 128 * (c + 1), 640:1024]
                )
            for c in range(4):  # wq o-blocks 1-3
                nc.scalar.dma_start(
                    out=wqkv[c][:, 128:512], in_=wqkv_d[128 * c : 128 * (c + 1), 128:512]
                )
            bvp = per.tile([128, 4], F32, tag="bvp")
            nc.gpsimd.dma_start(out=bvp[:], in_=bvp_d[:, :])
            wproj = []
            for c in range(4):
                w = per.tile([128, C], BF16, tag=f"wproj{c}", name=f"wproj{c}")
                nc.gpsimd.dma_start(out=w[:], in_=wproj_d[128 * c : 128 * (c + 1), :])
                wproj.append(w)
            bproj = per.tile([128, 4], F32, tag="bproj")
            nc.gpsimd.dma_start(out=bproj[:], in_=bproj_d[:, :])

            # ---- persistent activations ----
            qk = [per.tile([128, T], BF16, tag=f"qk{o}", name=f"qk{o}") for o in range(8)]
            vT = [per.tile([128, C], BF16, tag=f"vT{t}", name=f"vT{t}") for t in range(8)]
            attnout = [
                per.tile([128, T], BF16, tag=f"ao{p}", name=f"ao{p}") for p in range(4)
            ]
            out_sb = [
                per.tile([128, T], BF16, tag=f"os{o}", name=f"os{o}") for o in range(4)
            ]

            # ---------- emitters ----------
            qk_ps = {}
            # c-pass emission order matches the x DMA landing order
            corder = [0, 3, 1, 2]

            def qk_unit(o, half, c, first, last):
                # one c-pass of the (o, half) qk GEMM; accumulates in a pms
                # tile, bias-add+cast on VectorE at the last pass.
                tsl = slice(512 * half, 512 * (half + 1))
                if first:
                    qk_ps[(o, half)] = pms.tile(
                        [128, 512], F32, tag="ms", name=f"psqk{o}_{half}"
                    )
                ps = qk_ps[(o, half)]
                if o == 0:
                    lhsT = wcrit[:, 256 * c : 256 * c + 128]
                elif o == 4:
                    lhsT = wcrit[:, 256 * c + 128 : 256 * c + 256]
                else:
                    lhsT = wqkv[c][:, 128 * o : 128 * (o + 1)]
                nc.tensor.matmul(
                    ps[:],
                    lhsT=lhsT,
                    rhs=xs[c][:, tsl],
                    start=first,
                    stop=last,
                )
                if last:
                    nc.vector.tensor_scalar_add(qk[o][:, tsl], ps[:], bqk[:, o : o + 1])

            v_ps = {}

            def v_unit(tt, c):
                if c == 0:
                    v_ps[tt] = pms.tile([128, 512], F32, tag="ms", name=f"psv{tt}")
                ps = v_ps[tt]
                nc.tensor.matmul(
                    ps[:],
                    lhsT=xs[c][:, 128 * tt : 128 * (tt + 1)],
                    rhs=wqkv[c][:, 2 * C : 3 * C],
                    start=(c == 0),
                    stop=(c == 3),
                )
                if c == 3:
                    nc.vector.tensor_copy(vT[tt][:], ps[:])

            def sc_exp(p, th, u, g8):
                # scores chunk u (two heads row-packed) + exp on ScalarE
                tsl = slice(512 * th, 512 * (th + 1))
                ksl = slice(128 * u, 128 * (u + 1))
                ps = psc.tile([128, T], F32, tag="sc", name=f"pssc{p}_{th}_{u}")
                nc.tensor.matmul(
                    ps[:, 0:512],
                    lhsT=qk[4 + p][0:64, ksl], rhs=qk[p][0:64, tsl],
                    start=True, stop=True, tile_position=(0, 0),
                )
                nc.tensor.matmul(
                    ps[:, 512:1024],
                    lhsT=qk[4 + p][64:128, ksl], rhs=qk[p][64:128, tsl],
                    start=True, stop=True, tile_position=(64, 0),
                )
                P = pp.tile([128, T], BF16, tag=f"P{g8}{u % 2}",
                            name=f"P{p}_{th}_{u}")
                nc.scalar.activation(P[:], ps[:], AF.Exp)
                return P

            def av_chunk(p, u, P, psav):
                hA, hB = 2 * p, 2 * p + 1
                nc.tensor.matmul(
                    psav[0:64, :],
                    lhsT=vT[u][:, 64 * hA : 64 * (hA + 1)], rhs=P[:, 0:512],
                    start=(u == 0), stop=(u == 7), tile_position=(0, 0),
                )
                nc.tensor.matmul(
                    psav[64:128, :],
                    lhsT=vT[u][:, 64 * hB : 64 * (hB + 1)], rhs=P[:, 512:1024],
                    start=(u == 0), stop=(u == 7), tile_position=(0, 64),
                )

            def l4(j, Pe, Po, psl):
                # denominator partial sums for chunks (2j, 2j+1), both heads:
                # 4-way col-packed M=1 ones-matmuls. psl partitions:
                # 0 = headA/even-chunks, 32 = headA/odd, 64 = headB/even,
                # 96 = headB/odd; accumulate across the quarter's 4 blocks.
                st, sp = (j == 0), (j == 3)
                nc.tensor.matmul(
                    psl[0:1, :], lhsT=ones_all[:, 0:1], rhs=Pe[:, 0:512],
                    start=st, stop=sp, tile_position=(0, 0),
                )
                nc.tensor.matmul(
                    psl[64:65, :], lhsT=ones_all[:, 0:1], rhs=Pe[:, 512:1024],
                    start=st, stop=sp, tile_position=(0, 64),
                )
                nc.tensor.matmul(
                    psl[32:33, :], lhsT=ones_all[:, 0:1], rhs=Po[:, 0:512],
                    start=st, stop=sp, tile_position=(0, 32),
                )
                nc.tensor.matmul(
                    psl[96:97, :], lhsT=ones_all[:, 0:1], rhs=Po[:, 512:1024],
                    start=st, stop=sp, tile_position=(0, 96),
                )

            def finalize_head(p, th, psav, psl):
                # free pav / psl as early as possible: ScalarE copies run in
                # the natural exp-stream bubble at the quarter boundary, so
                # the next quarter's av/l4 matmuls don't queue on VectorE.
                avc = scr.tile([128, 512], F32, tag="avc", name=f"avc{p}_{th}")
                nc.scalar.copy(avc[:], psav[:])
                lbq = scr.tile([128, 512], BF16, tag="lb", name=f"lb{p}_{th}")
                nc.vector.tensor_copy(lbq[:], psl[:])
                return avc, lbq

            def finalize_tail(p, th, avc, lbq):
                tsl = slice(512 * th, 512 * (th + 1))
                # broadcast-and-parity-sum: K=33 ones-matmuls over padded row
                # ranges of lbq (rows 1-31, 33-63, 65-95, 97-127 are zero).
                psbc = pms.tile([128, 512], F32, tag="ms", name=f"psbc{p}_{th}")
                nc.tensor.matmul(
                    psbc[0:64, :], lhsT=ones_all[0:33, 0:64], rhs=lbq[0:33, :],
                    start=True, stop=True, tile_position=(0, 0),
                )
                nc.tensor.matmul(
                    psbc[64:128, :], lhsT=ones_all[64:97, 0:64], rhs=lbq[64:97, :],
                    start=True, stop=True, tile_position=(64, 64),
                )
                bc = scr.tile([128, 512], F32, tag="bc", name=f"bc{p}_{th}")
                nc.vector.reciprocal_approx_fast(bc[:], psbc[:])
                tmpn = scr.tile([128, 512], F32, tag="tmpn", name=f"tn{p}_{th}")
                nc.vector.tensor_mul(tmpn[:], avc[:], bc[:])
                nc.vector.tensor_scalar_add(
                    attnout[p][:, tsl], tmpn[:], bvp[:, p : p + 1]
                )

            proj_ps = {}

            def proj_unit(o, th, c):
                tsl = slice(512 * th, 512 * (th + 1))
                if c == 0:
                    proj_ps[(o, th)] = pms.tile(
                        [128, 512], F32, tag="ms", name=f"pspr{o}_{th}"
                    )
                ps = proj_ps[(o, th)]
                nc.tensor.matmul(
                    ps[:],
                    lhsT=wproj[c][:, 128 * o : 128 * (o + 1)],
                    rhs=attnout[c][:, tsl],
                    start=(c == 0),
                    stop=(c == 3),
                )
                if c == 3:
                    nc.vector.tensor_scalar_add(
                        out_sb[o][:, tsl], ps[:], bproj[:, o : o + 1]
                    )
                    eng = [nc.sync, nc.gpsimd, nc.scalar, nc.sync][o]
                    eng.dma_start(
                        out=out_d[128 * o : 128 * (o + 1), tsl], in_=out_sb[o][:, tsl]
                    )

            # ---------- static filler schedule ----------
            def qk_full(o, half):
                return [
                    lambda c=c, i=i: qk_unit(o, half, c, i == 0, i == 3)
                    for i, c in enumerate(corder)
                ]

            def v_full(tt):
                return [lambda c=c: v_unit(tt, c) for c in range(4)]

            def proj_full(o, th):
                return [lambda c=c: proj_unit(o, th, c) for c in range(4)]

            # fillers[b] = (pre_av_units, post_av_units). Deadlines:
            #   vT[u] before the av that consumes it (av at block b covers
            #   chunks of pair b-LAG); qk halves before the sc that reads
            #   them; proj(:,0) after finalize(q3) pops at block 21.
            fillers = {
                0: ([], qk_full(4, 1)[:2] + v_full(0)),
                1: ([], qk_full(4, 1)[2:] + v_full(1)),
                2: ([], v_full(2) + qk_full(5, 0)[:2]),
                3: ([], qk_full(5, 0)[2:] + qk_full(1, 0)),
                4: (v_full(3), v_full(4) + qk_full(5, 1)[:2]),
                5: (v_full(5), qk_full(5, 1)[2:] + v_full(6)[:2]),
                6: (v_full(6)[2:] + v_full(7), qk_full(6, 0)[:2]),
                7: ([], qk_full(6, 0)[2:] + qk_full(2, 0)),
                8: ([], qk_full(6, 1)[:2]),
                9: ([], qk_full(6, 1)[2:] + qk_full(7, 0)[:2]),
                10: ([], qk_full(7, 0)[2:] + qk_full(3, 0)[:2]),
                11: ([], qk_full(3, 0)[2:] + qk_full(7, 1)[:2]),
                12: ([], qk_full(7, 1)[2:] + qk_full(0, 1)[:2]),
                13: ([], qk_full(0, 1)[2:] + qk_full(1, 1)[:2]),
                14: ([], qk_full(1, 1)[2:] + qk_full(2, 1)[:2]),
                15: ([], qk_full(2, 1)[2:] + qk_full(3, 1)[:2]),
                16: ([], qk_full(3, 1)[2:]),
                19: ([], proj_full(0, 0)),
                20: ([], proj_full(1, 0)),
                21: ([], proj_full(2, 0)),
                22: ([], proj_full(3, 0)),
            }

            # ---------- startup PE work ----------
            # single persistent psl tile; zero its never-written partitions
            # once so the padded K=33 broadcast reads are defined.
            psl_t = pslp.tile([128, 512], F32, tag="l", name="psl")
            nc.vector.memset(psl_t[:], 0.0)
            for f in qk_full(0, 0) + qk_full(4, 0):
                f()

            # ---------- main pipeline ----------
            LAG = 3
            quarters = [(p, th) for th in range(2) for p in range(4)]
            pend = []          # (p, th, j, P_even, P_odd)
            qtiles = {}        # (p, th) -> psav

            fin_q = []

            def emit_avl(ent):
                p, th, j, Pe, Po = ent
                if (p, th) not in qtiles:
                    qtiles[(p, th)] = pav.tile(
                        [128, 512], F32, tag="av", name=f"psav{p}_{th}"
                    )
                psav = qtiles[(p, th)]
                av_chunk(p, 2 * j, Pe, psav)
                av_chunk(p, 2 * j + 1, Po, psav)
                l4(j, Pe, Po, psl_t)
                if j == 3:
                    avc, lbq = finalize_head(p, th, psav, psl_t)
                    fin_q.append(lambda a=avc, b=lbq, pp_=p, tt=th:
                                 finalize_tail(pp_, tt, a, b))

            # drain the av/l pipeline early over the last quarters' blocks so
            # the tail holds only the final finalize + proj(:,1).
            drain_target = {28: 3, 29: 2, 30: 1, 31: 0}
            for b in range(32):
                qi, j = b // 4, b % 4
                p, th = quarters[qi]
                pre, post = fillers.get(b, ([], []))
                for f in pre:
                    f()
                Pe = sc_exp(p, th, 2 * j, b % 4)
                Po = sc_exp(p, th, 2 * j + 1, b % 4)
                pend.append((p, th, j, Pe, Po))
                while fin_q:
                    fin_q.pop(0)()
                while len(pend) > drain_target.get(b, LAG):
                    emit_avl(pend.pop(0))
                for f in post:
                    f()
            while pend:
                emit_avl(pend.pop(0))
            while fin_q:
                fin_q.pop(0)()
            for o in range(4):
                for f in proj_full(o, 1):
                    f()

    nc.compile()
    return nc


def _get_nc():
    if "nc" not in _CACHE:
        _CACHE["nc"] = _build()
    return _CACHE["nc"]


def kernel(x, qkv_w, qkv_b, proj_w, proj_b, _trace=False):
    from concourse.bass_utils import run_bass_kernel_spmd

    nc = _get_nc()

    bf16 = ml_dtypes.bfloat16
    b, c, h, w = x.shape
    xf = np.asarray(x, dtype=np.float32).reshape(b, c, h * w)
    qkv_b = np.asarray(qkv_b, dtype=np.float32)
    qkv_w = np.asarray(qkv_w, dtype=np.float32)
    # fold the 1/sqrt(e)=1/8 softmax scale into Wq / bq on host
    qkv_w = np.concatenate([qkv_w[:512] * 0.125, qkv_w[512:]], axis=0)
    bq = np.concatenate([qkv_b[:512] * 0.125, qkv_b[512:1024]])
    wqkvT = np.ascontiguousarray(qkv_w.T).astype(bf16)
    wprojT = np.ascontiguousarray(np.asarray(proj_w, np.float32).T).astype(bf16)
    bqk = np.ascontiguousarray(bq.reshape(8, 128).T)
    bvp = np.ascontiguousarray(qkv_b[1024:1536].reshape(4, 128).T)
    bproj = np.ascontiguousarray(np.asarray(proj_b, np.float32).reshape(4, 128).T)

    wcrit = np.concatenate(
        [
            np.concatenate(
                [wqkvT[128 * c : 128 * (c + 1), 0:128],
                 wqkvT[128 * c : 128 * (c + 1), 512:640]],
                axis=1,
            )
            for c in range(4)
        ],
        axis=1,
    )
    wcrit = np.ascontiguousarray(wcrit)

    in_maps = [
        dict(
            x=np.ascontiguousarray(xf[i]).astype(bf16),
            wqkv=wqkvT,
            wcrit=wcrit,
            wproj=wprojT,
            bqk=bqk,
            bvp=bvp,
            bproj=bproj,
        )
        for i in range(b)
    ]
    res = run_bass_kernel_spmd(nc, in_maps, core_ids=list(range(8)), trace=_trace)
    out = np.stack([res.results[i]["out"].astype(np.float32) for i in range(b)])
    out = out.reshape(b, c, h, w)
    if _trace:
        _CACHE["last_result"] = res
    return out
